# revision 7
# baseline (speedup 1.0000x reference)
"""DLDMD kernel for Trainium2 (8 NeuronCores, batch-sharded).

Device (Bass/Tile, SPMD over 8 cores, 64 trajectories each):
  - encoder MLP   x  [B,T,3]  -> y     [B,T,32]   ("pair" mode: 3-product
    float32r hi/lo matmuls, fp32-grade accuracy at 3 cyc/row)
  - decoder MLP   y          -> x_ae   [B,T,3]    (plain float32r matmuls)
  - decoder MLP   y_adv      -> x_adv  [B,P,3]    (plain float32r matmuls)
Host (jnp on CPU, replicating the reference's EDMD chain op-for-op):
  - SVD -> A -> eig -> phi -> Vandermonde powers -> y_adv
  (complex nonsymmetric eig has no Trainium implementation; the chain is
  numerically chaotic so it must be replicated with the identical LAPACK
  calls the reference uses, seeded by the device-computed y.)

"pair" mode: weights/activations are split on-device into a float32r
value plus a float32r residual (W = Wr + Wd, h = hr + hd); the product
is computed as Wr.hr + Wr.hd + Wd.hr, three 1-cycle/row f32r matmuls
accumulated in PSUM, recovering ~24-bit precision (measured 5e-7 vs
plain fp32's 4 cycle/row path).
"""

import numpy as np

B, T, P_STEPS = 512, 256, 256
PHYS, LAT, NEUR, NLAYERS = 3, 32, 256, 4
N_CORES = 8
BPC = B // N_CORES              # trajectories per core
TOK_ENC = BPC * T               # encoder tokens per core
TOK_DEC = 2 * TOK_ENC           # decoder tokens per core (y ++ y_adv)
SC = 1024                       # tokens per superchunk (2 psum banks per tile)
MM = 512                        # moving-operand free size per matmul (fp32 max)
P = 128

# weight-pack column layout (one [128, WCOLS] array, single DMA)
_W_IN0 = 0                       # w_in   rows 0:Fin        cols [0, NEUR)
_W_H0 = NEUR                     # w_h    (i,k) -> NEUR cols each
_W_OUT0 = _W_H0 + NLAYERS * 2 * NEUR
def _pack_cols(fout):
    w_out0 = _W_OUT0
    b_in0 = w_out0 + 2 * fout
    b_h0 = b_in0 + 2
    b_out0 = b_h0 + NLAYERS * 2
    return w_out0, b_in0, b_h0, b_out0, b_out0 + 1


def _pack_weights(w_in, b_in, w_h, b_h, w_out, b_out):
    fin, fout = w_in.shape[0], w_out.shape[1]
    w_out0, b_in0, b_h0, b_out0, wcols = _pack_cols(fout)
    pk = np.zeros((P, wcols), np.float32)
    pk[0:fin, _W_IN0:_W_IN0 + NEUR] = w_in
    for i in range(NLAYERS):
        for k in range(2):
            pk[:, _W_H0 + (i * 2 + k) * NEUR: _W_H0 + (i * 2 + k + 1) * NEUR] = \
                w_h[i, k * P:(k + 1) * P, :]
    for k in range(2):
        pk[:, w_out0 + k * fout: w_out0 + (k + 1) * fout] = w_out[k * P:(k + 1) * P, :]
    pk[:, b_in0] = b_in[0:P]
    pk[:, b_in0 + 1] = b_in[P:NEUR]
    for i in range(NLAYERS):
        for m in range(2):
            pk[:, b_h0 + i * 2 + m] = b_h[i, m * P:(m + 1) * P]
    pk[0:fout, b_out0] = b_out
    return pk


def _build_mlp_module(fin, fout, ntok, mode):
    """One SPMD module: xt [fin, ntok] -> yt [fout, ntok] through the MLP.

    mode: "f32r" (1 matmul/K-tile, ~5e-4/layer) or "pair" (3 f32r
    matmuls/K-tile with hi/lo residuals, fp32-grade accuracy).
    """
    import concourse.bacc as bacc
    import concourse.tile as tile
    import concourse.mybir as mybir

    F32 = mybir.dt.float32
    F32R = mybir.dt.float32r
    AFT = mybir.ActivationFunctionType
    pair = mode == "pair"
    IODT = F32 if pair else F32R    # dram/tile dtype for raw inputs + weights

    w_out0, b_in0, b_h0, b_out0, wcols = _pack_cols(fout)
    wend = w_out0 + 2 * fout        # weight region (excl. biases)

    nc = bacc.Bacc("TRN2", target_bir_lowering=False, debug=False,
                   num_devices=N_CORES)
    xt_d = nc.dram_tensor("xt", [fin, ntok], IODT, kind="ExternalInput").ap()
    w_d = nc.dram_tensor("wpack", [P, wcols], IODT, kind="ExternalInput").ap()
    yt_d = nc.dram_tensor("yt", [fout, ntok], F32, kind="ExternalOutput").ap()

    nsc = ntok // SC
    GRP = 2  # superchunks interleaved per emission wave
    with tile.TileContext(nc) as tc:
        with tc.tile_pool(name="wp", bufs=1) as wp, \
             tc.tile_pool(name="ap", bufs=3) as apool, \
             tc.tile_pool(name="hp", bufs=4 if pair else 12) as hpool, \
             tc.tile_pool(name="hrp", bufs=14) as hrpool, \
             tc.tile_pool(name="op", bufs=4) as opool, \
             tc.tile_pool(name="ps", bufs=3, space="PSUM") as psp, \
             tc.tile_pool(name="pso", bufs=2, space="PSUM") as psop:
            ws = wp.tile([P, wcols], IODT)
            nc.sync.dma_start(ws[:], w_d[:, :])
            if pair:
                wr = wp.tile([P, wend], F32R)
                wd = wp.tile([P, wend], F32R)
                nc.gpsimd.tensor_copy(wr[:], ws[:, 0:wend])
                nc.vector.tensor_sub(wd[:], ws[:, 0:wend], wr[:].bitcast(F32))
            else:
                wr, wd = ws, None
            # dummy matmuls: absorb the weight-producer waits on PE so every
            # real matmul needs at most one sync wait (LDWEIGHTS allows one).
            dps = psop.tile([1, 1], F32, tag="pso", name="dummy_ps")
            nc.tensor.matmul(dps[0:1, 0:1], wr[:, 0:1].bitcast(F32),
                             wr[:, 1:2].bitcast(F32), start=True, stop=True)
            if pair:
                dps2 = psop.tile([1, 1], F32, tag="pso", name="dummy_ps2")
                nc.tensor.matmul(dps2[0:1, 0:1], wd[:, 0:1].bitcast(F32),
                                 wd[:, 1:2].bitcast(F32), start=True, stop=True)

            def bias(col, rows=P):
                return ws[0:rows, col:col + 1].bitcast(F32)

            def products(w0, w1, rhs):
                """matmul operand pairs for one K-tile: weights cols
                [w0:w1], rhs = (value, residual-or-None) slices."""
                r, d = rhs
                if not pair:
                    return [(wr[:, w0:w1], r)]
                return [(wr[:, w0:w1], r), (wr[:, w0:w1], d), (wd[:, w0:w1], r)]

            def accumulate(ps_slice, ktiles):
                """ktiles: list of (w0, w1, krows, rhs) accumulated into ps."""
                ops = []
                for (w0, w1, kr, rhs) in ktiles:
                    for wsl, rsl in products(w0, w1, rhs):
                        ops.append((wsl[0:kr, :], rsl))
                for idx, (wsl, rsl) in enumerate(ops):
                    nc.tensor.matmul(ps_slice, wsl, rsl,
                                     start=(idx == 0), stop=(idx == len(ops) - 1))

            def make_pair(c, tag, src, fdim):
                """round src (fp32) to f32r + residual, on GpSimd + DVE."""
                r = hrpool.tile([fdim, SC], F32R, tag="pr", name=f"{tag}r{c}")
                d = hrpool.tile([fdim, SC], F32R, tag="pd", name=f"{tag}d{c}")
                nc.gpsimd.tensor_copy(r[:], src[:])
                nc.vector.tensor_sub(d[:], src[:], r[:].bitcast(F32))
                return (r, d)

            def act_to_pair(c, label, ps, biascol):
                """tanh(ps + bias) -> (value, residual) in matmul dtype."""
                if not pair:
                    h = hpool.tile([P, SC], F32R, tag="h", name=f"h{c}_{label}")
                    nc.scalar.activation(h[:], ps[:], AFT.Tanh, bias=bias(biascol))
                    return (h, None)
                h32 = hpool.tile([P, SC], F32, tag="h32", name=f"h32{c}_{label}")
                nc.scalar.activation(h32[:], ps[:], AFT.Tanh, bias=bias(biascol))
                return make_pair(c, f"h_{label}_", h32, P)

            def in_layer(c, a):
                h = []
                for m in range(2):
                    ps = psp.tile([P, SC], F32, tag="ps", name=f"ps{c}_in{m}")
                    for j in range(SC // MM):
                        jj = slice(j * MM, (j + 1) * MM)
                        accumulate(ps[:, jj],
                                   [(_W_IN0 + m * P, _W_IN0 + (m + 1) * P, fin,
                                     tuple(t[:, jj] if t is not None else None
                                           for t in a))])
                    h.append(act_to_pair(c, f"in{m}", ps, b_in0 + m))
                return h

            def hidden_layer(c, i, h):
                h2 = []
                for m in range(2):
                    ps = psp.tile([P, SC], F32, tag="ps", name=f"ps{c}_l{i}_{m}")
                    for j in range(SC // MM):
                        jj = slice(j * MM, (j + 1) * MM)
                        accumulate(ps[:, jj], [
                            (_W_H0 + (i * 2 + k) * NEUR + m * P,
                             _W_H0 + (i * 2 + k) * NEUR + (m + 1) * P, P,
                             tuple(t[:, jj] if t is not None else None
                                   for t in h[k]))
                            for k in range(2)])
                    h2.append(act_to_pair(c, f"l{i}_{m}", ps, b_h0 + i * 2 + m))
                return h2

            def out_layer(c, h):
                for j in range(SC // MM):
                    jj = slice(j * MM, (j + 1) * MM)
                    pso = psop.tile([fout, MM], F32, tag="pso", name=f"pso{c}_{j}")
                    accumulate(pso[:], [
                        (w_out0 + k * fout, w_out0 + (k + 1) * fout, P,
                         tuple(t[:, jj] if t is not None else None
                               for t in h[k]))
                        for k in range(2)])
                    o = opool.tile([fout, MM], F32, tag="o", name=f"o{c}_{j}")
                    nc.vector.tensor_scalar_add(o[:], pso[:], bias(b_out0, fout))
                    nc.sync.dma_start(
                        yt_d[:, c * SC + j * MM: c * SC + (j + 1) * MM], o[:])

            for c0 in range(0, nsc, GRP):
                subs = range(c0, min(c0 + GRP, nsc))
                avs = []
                for c in subs:
                    a = apool.tile([fin, SC], IODT, tag="a", name=f"a{c}")
                    nc.sync.dma_start(a[:], xt_d[:, c * SC:(c + 1) * SC])
                    avs.append(make_pair(c, "a_", a, fin) if pair else (a, None))
                hs = [in_layer(c, a) for c, a in zip(subs, avs)]
                for i in range(NLAYERS):
                    hs = [hidden_layer(c, i, h) for c, h in zip(subs, hs)]
                for c, h in zip(subs, hs):
                    out_layer(c, h)
    nc.compile()
    return nc


_MODULE_CACHE = {}


def _get_module(key, builder):
    if key not in _MODULE_CACHE:
        _MODULE_CACHE[key] = builder()
    return _MODULE_CACHE[key]


def _run_spmd(nc, in_maps):
    from concourse import bass_utils
    res = bass_utils.run_bass_kernel_spmd(nc, in_maps,
                                          core_ids=list(range(N_CORES)))
    return res.results


def kernel(x, enc_W_in, enc_b_in, enc_W_h, enc_b_h, enc_W_out, enc_b_out,
           dec_W_in, dec_b_in, dec_W_h, dec_b_h, dec_W_out, dec_b_out):
    x = np.ascontiguousarray(np.asarray(x, np.float32))

    enc_pack = _pack_weights(np.asarray(enc_W_in), np.asarray(enc_b_in),
                             np.asarray(enc_W_h), np.asarray(enc_b_h),
                             np.asarray(enc_W_out), np.asarray(enc_b_out))
    dec_pack = _pack_weights(np.asarray(dec_W_in), np.asarray(dec_b_in),
                             np.asarray(dec_W_h), np.asarray(dec_b_h),
                             np.asarray(dec_W_out), np.asarray(dec_b_out))

    enc_nc = _get_module("enc", lambda: _build_mlp_module(PHYS, LAT, TOK_ENC, "pair"))
    dec_nc = _get_module("dec", lambda: _build_mlp_module(LAT, PHYS, TOK_DEC, "f32r"))

    # ---- launch 1: encoder ----
    in_maps = []
    for c in range(N_CORES):
        xs = x[c * BPC:(c + 1) * BPC].reshape(TOK_ENC, PHYS)
        in_maps.append({"xt": np.ascontiguousarray(xs.T), "wpack": enc_pack})
    enc_res = _run_spmd(enc_nc, in_maps)
    y = np.concatenate(
        [enc_res[c]["yt"].T.reshape(BPC, T, LAT) for c in range(N_CORES)], axis=0)

    # ---- host: EDMD chain, replicated verbatim from the reference ----
    import jax
    import jax.numpy as jnp
    cpu = jax.devices("cpu")[0]
    with jax.default_device(cpu):
        yj = jnp.asarray(y)
        Y = jnp.swapaxes(yj, 1, 2)
        Y_m = Y[:, :, :-1]
        Y_p = Y[:, :, 1:]
        U, sig, Vh = jnp.linalg.svd(Y_m, full_matrices=False)
        A = ((Y_p @ jnp.swapaxes(Vh, -1, -2)) * (1.0 / sig)[:, None, :]) \
            @ jnp.swapaxes(U, -1, -2)
        evals, evecs = jnp.linalg.eig(A)
        phi = jnp.linalg.solve(evecs, Y_m.astype(evecs.dtype))
        y0 = phi[:, :, 0]
        powers = evals[:, None, :] ** jnp.arange(P_STEPS)[None, :, None]
        y_adv = jnp.real(jnp.einsum('blm,bkm->bkl', evecs, powers * y0[:, None, :]))
        evals = np.asarray(evals)
        evecs = np.asarray(evecs)
        phi = np.asarray(phi)
        y_adv = np.asarray(y_adv)

    # ---- launch 2: decoder on [y ; y_adv] ----
    in_maps = []
    for c in range(N_CORES):
        z = np.empty((TOK_DEC, LAT), np.float32)
        z[:TOK_ENC] = y[c * BPC:(c + 1) * BPC].reshape(TOK_ENC, LAT)
        z[TOK_ENC:] = y_adv[c * BPC:(c + 1) * BPC].reshape(TOK_ENC, LAT)
        in_maps.append({"xt": np.ascontiguousarray(z.T), "wpack": dec_pack})
    dec_res = _run_spmd(dec_nc, in_maps)
    x_ae = np.concatenate(
        [dec_res[c]["yt"][:, :TOK_ENC].T.reshape(BPC, T, PHYS)
         for c in range(N_CORES)], axis=0)
    x_adv = np.concatenate(
        [dec_res[c]["yt"][:, TOK_ENC:].T.reshape(BPC, P_STEPS, PHYS)
         for c in range(N_CORES)], axis=0)

    return (y, x_ae, x_adv, y_adv, evals, evecs, phi)


# revision 9
# speedup vs baseline: 1.3355x; 1.3355x over previous
"""DLDMD kernel for Trainium2 (8 NeuronCores, batch-sharded).

Device (Bass/Tile, SPMD over 8 cores, 64 trajectories each):
  - encoder MLP   x  [B,T,3]  -> y     [B,T,32]   ("pair" mode: 3-product
    float32r hi/lo matmuls, fp32-grade accuracy at 3 cyc/row)
  - decoder MLP   y          -> x_ae   [B,T,3]    (plain float32r matmuls)
  - decoder MLP   y_adv      -> x_adv  [B,P,3]    (plain float32r matmuls)
Host (jnp on CPU, replicating the reference's EDMD chain op-for-op):
  - SVD -> A -> eig -> phi -> Vandermonde powers -> y_adv
  (complex nonsymmetric eig has no Trainium implementation; the chain is
  numerically chaotic so it must be replicated with the identical LAPACK
  calls the reference uses, seeded by the device-computed y.)

"pair" mode: weights/activations are split on-device into a float32r
value plus a float32r residual (W = Wr + Wd, h = hr + hd); the product
is computed as Wr.hr + Wr.hd + Wd.hr, three 1-cycle/row f32r matmuls
accumulated in PSUM, recovering ~24-bit precision (measured 5e-7 vs
plain fp32's 4 cycle/row path).
"""

import numpy as np

B, T, P_STEPS = 512, 256, 256
PHYS, LAT, NEUR, NLAYERS = 3, 32, 256, 4
N_CORES = 8
BPC = B // N_CORES              # trajectories per core
TOK_ENC = BPC * T               # encoder tokens per core
TOK_DEC = 2 * TOK_ENC           # decoder tokens per core (y ++ y_adv)
SC = 1024                       # tokens per superchunk (2 psum banks per tile)
MM = 512                        # moving-operand free size per matmul (fp32 max)
P = 128

# weight-pack column layout (one [128, WCOLS] array, single DMA)
_W_IN0 = 0                       # w_in   rows 0:Fin        cols [0, NEUR)
_W_H0 = NEUR                     # w_h    (i,k) -> NEUR cols each
_W_OUT0 = _W_H0 + NLAYERS * 2 * NEUR
def _pack_cols(fout):
    w_out0 = _W_OUT0
    b_in0 = w_out0 + 2 * fout
    b_h0 = b_in0 + 2
    b_out0 = b_h0 + NLAYERS * 2
    return w_out0, b_in0, b_h0, b_out0, b_out0 + 1


def _pack_weights(w_in, b_in, w_h, b_h, w_out, b_out):
    fin, fout = w_in.shape[0], w_out.shape[1]
    w_out0, b_in0, b_h0, b_out0, wcols = _pack_cols(fout)
    pk = np.zeros((P, wcols), np.float32)
    pk[0:fin, _W_IN0:_W_IN0 + NEUR] = w_in
    for i in range(NLAYERS):
        for k in range(2):
            pk[:, _W_H0 + (i * 2 + k) * NEUR: _W_H0 + (i * 2 + k + 1) * NEUR] = \
                w_h[i, k * P:(k + 1) * P, :]
    for k in range(2):
        pk[:, w_out0 + k * fout: w_out0 + (k + 1) * fout] = w_out[k * P:(k + 1) * P, :]
    pk[:, b_in0] = b_in[0:P]
    pk[:, b_in0 + 1] = b_in[P:NEUR]
    for i in range(NLAYERS):
        for m in range(2):
            pk[:, b_h0 + i * 2 + m] = b_h[i, m * P:(m + 1) * P]
    pk[0:fout, b_out0] = b_out
    return pk


def _build_mlp_module(fin, fout, ntok, mode):
    """One SPMD module: xt [fin, ntok] -> yt [fout, ntok] through the MLP.

    mode: "f32r" (1 matmul/K-tile, ~5e-4/layer) or "pair" (3 f32r
    matmuls/K-tile with hi/lo residuals, fp32-grade accuracy).
    """
    import concourse.bacc as bacc
    import concourse.tile as tile
    import concourse.mybir as mybir

    F32 = mybir.dt.float32
    F32R = mybir.dt.float32r
    AFT = mybir.ActivationFunctionType
    pair = mode == "pair"
    IODT = F32 if pair else F32R    # dram/tile dtype for raw inputs + weights

    w_out0, b_in0, b_h0, b_out0, wcols = _pack_cols(fout)
    wend = w_out0 + 2 * fout        # weight region (excl. biases)

    nc = bacc.Bacc("TRN2", target_bir_lowering=False, debug=False,
                   num_devices=N_CORES)
    xt_d = nc.dram_tensor("xt", [fin, ntok], IODT, kind="ExternalInput").ap()
    w_d = nc.dram_tensor("wpack", [P, wcols], IODT, kind="ExternalInput").ap()
    yt_d = nc.dram_tensor("yt", [fout, ntok], F32, kind="ExternalOutput").ap()

    nsc = ntok // SC
    GRP = 2  # superchunks interleaved per emission wave
    with tile.TileContext(nc) as tc:
        with tc.tile_pool(name="wp", bufs=1) as wp, \
             tc.tile_pool(name="ap", bufs=3) as apool, \
             tc.tile_pool(name="hp", bufs=4 if pair else 12) as hpool, \
             tc.tile_pool(name="hrp", bufs=14) as hrpool, \
             tc.tile_pool(name="op", bufs=4) as opool, \
             tc.tile_pool(name="ps", bufs=3, space="PSUM") as psp, \
             tc.tile_pool(name="pso", bufs=2, space="PSUM") as psop:
            ws = wp.tile([P, wcols], IODT)
            nc.sync.dma_start(ws[:], w_d[:, :])
            if pair:
                wr = wp.tile([P, wend], F32R)
                wd = wp.tile([P, wend], F32R)
                nc.gpsimd.tensor_copy(wr[:], ws[:, 0:wend])
                nc.vector.tensor_sub(wd[:], ws[:, 0:wend], wr[:].bitcast(F32))
            else:
                wr, wd = ws, None
            # dummy matmuls: absorb the weight-producer waits on PE so every
            # real matmul needs at most one sync wait (LDWEIGHTS allows one).
            dps = psop.tile([1, 1], F32, tag="pso", name="dummy_ps")
            nc.tensor.matmul(dps[0:1, 0:1], wr[:, 0:1].bitcast(F32),
                             wr[:, 1:2].bitcast(F32), start=True, stop=True)
            if pair:
                dps2 = psop.tile([1, 1], F32, tag="pso", name="dummy_ps2")
                nc.tensor.matmul(dps2[0:1, 0:1], wd[:, 0:1].bitcast(F32),
                                 wd[:, 1:2].bitcast(F32), start=True, stop=True)

            def bias(col, rows=P):
                return ws[0:rows, col:col + 1].bitcast(F32)

            def products(w0, w1, rhs):
                """matmul operand pairs for one K-tile: weights cols
                [w0:w1], rhs = (value, residual-or-None) slices."""
                r, d = rhs
                if not pair:
                    return [(wr[:, w0:w1], r)]
                return [(wr[:, w0:w1], r), (wd[:, w0:w1], r), (wr[:, w0:w1], d)]

            def accumulate(ps_slice, ktiles):
                """ktiles: list of (w0, w1, krows, rhs) accumulated into ps."""
                ops = []
                for (w0, w1, kr, rhs) in ktiles:
                    for wsl, rsl in products(w0, w1, rhs):
                        ops.append((wsl[0:kr, :], rsl))
                for idx, (wsl, rsl) in enumerate(ops):
                    nc.tensor.matmul(ps_slice, wsl, rsl,
                                     start=(idx == 0), stop=(idx == len(ops) - 1))

            def make_pair(c, tag, src, fdim):
                """round src (fp32) to f32r + residual, on GpSimd + DVE."""
                r = hrpool.tile([fdim, SC], F32R, tag="pr", name=f"{tag}r{c}")
                d = hrpool.tile([fdim, SC], F32R, tag="pd", name=f"{tag}d{c}")
                nc.gpsimd.tensor_copy(r[:], src[:])
                nc.vector.tensor_sub(d[:], src[:], r[:].bitcast(F32))
                return (r, d)

            def act_to_pair(c, label, ps, biascol):
                """tanh(ps + bias) -> (value, residual) in matmul dtype.

                pair mode: ACT evaluates tanh twice — once rounded to f32r
                (hr, feeds PE immediately) and once in fp32 (h32, only for
                the DVE residual hd = h32 - hr)."""
                if not pair:
                    h = hpool.tile([P, SC], F32R, tag="h", name=f"h{c}_{label}")
                    nc.scalar.activation(h[:], ps[:], AFT.Tanh, bias=bias(biascol))
                    return (h, None)
                hr = hrpool.tile([P, SC], F32R, tag="pr", name=f"hr{c}_{label}")
                nc.scalar.activation(hr[:], ps[:], AFT.Tanh, bias=bias(biascol))
                h32 = hpool.tile([P, SC], F32, tag="h32", name=f"h32{c}_{label}")
                nc.scalar.activation(h32[:], ps[:], AFT.Tanh, bias=bias(biascol))
                hd = hrpool.tile([P, SC], F32R, tag="pd", name=f"hd{c}_{label}")
                nc.vector.tensor_sub(hd[:], h32[:], hr[:].bitcast(F32))
                return (hr, hd)

            def in_layer(c, a):
                h = []
                for m in range(2):
                    ps = psp.tile([P, SC], F32, tag="ps", name=f"ps{c}_in{m}")
                    for j in range(SC // MM):
                        jj = slice(j * MM, (j + 1) * MM)
                        accumulate(ps[:, jj],
                                   [(_W_IN0 + m * P, _W_IN0 + (m + 1) * P, fin,
                                     tuple(t[:, jj] if t is not None else None
                                           for t in a))])
                    h.append(act_to_pair(c, f"in{m}", ps, b_in0 + m))
                return h

            def hidden_layer(c, i, h):
                h2 = []
                for m in range(2):
                    ps = psp.tile([P, SC], F32, tag="ps", name=f"ps{c}_l{i}_{m}")
                    for j in range(SC // MM):
                        jj = slice(j * MM, (j + 1) * MM)
                        accumulate(ps[:, jj], [
                            (_W_H0 + (i * 2 + k) * NEUR + m * P,
                             _W_H0 + (i * 2 + k) * NEUR + (m + 1) * P, P,
                             tuple(t[:, jj] if t is not None else None
                                   for t in h[k]))
                            for k in range(2)])
                    h2.append(act_to_pair(c, f"l{i}_{m}", ps, b_h0 + i * 2 + m))
                return h2

            def out_layer(c, h):
                for j in range(SC // MM):
                    jj = slice(j * MM, (j + 1) * MM)
                    pso = psop.tile([fout, MM], F32, tag="pso", name=f"pso{c}_{j}")
                    accumulate(pso[:], [
                        (w_out0 + k * fout, w_out0 + (k + 1) * fout, P,
                         tuple(t[:, jj] if t is not None else None
                               for t in h[k]))
                        for k in range(2)])
                    o = opool.tile([fout, MM], F32, tag="o", name=f"o{c}_{j}")
                    nc.vector.tensor_scalar_add(o[:], pso[:], bias(b_out0, fout))
                    nc.sync.dma_start(
                        yt_d[:, c * SC + j * MM: c * SC + (j + 1) * MM], o[:])

            for c0 in range(0, nsc, GRP):
                subs = range(c0, min(c0 + GRP, nsc))
                avs = []
                for c in subs:
                    a = apool.tile([fin, SC], IODT, tag="a", name=f"a{c}")
                    nc.sync.dma_start(a[:], xt_d[:, c * SC:(c + 1) * SC])
                    avs.append(make_pair(c, "a_", a, fin) if pair else (a, None))
                hs = [in_layer(c, a) for c, a in zip(subs, avs)]
                for i in range(NLAYERS):
                    hs = [hidden_layer(c, i, h) for c, h in zip(subs, hs)]
                for c, h in zip(subs, hs):
                    out_layer(c, h)
    nc.compile()
    return nc


_MODULE_CACHE = {}


def _get_module(key, builder):
    if key not in _MODULE_CACHE:
        _MODULE_CACHE[key] = builder()
    return _MODULE_CACHE[key]


def _run_spmd(nc, in_maps):
    from concourse import bass_utils
    res = bass_utils.run_bass_kernel_spmd(nc, in_maps,
                                          core_ids=list(range(N_CORES)))
    return res.results


def kernel(x, enc_W_in, enc_b_in, enc_W_h, enc_b_h, enc_W_out, enc_b_out,
           dec_W_in, dec_b_in, dec_W_h, dec_b_h, dec_W_out, dec_b_out):
    x = np.ascontiguousarray(np.asarray(x, np.float32))

    enc_pack = _pack_weights(np.asarray(enc_W_in), np.asarray(enc_b_in),
                             np.asarray(enc_W_h), np.asarray(enc_b_h),
                             np.asarray(enc_W_out), np.asarray(enc_b_out))
    dec_pack = _pack_weights(np.asarray(dec_W_in), np.asarray(dec_b_in),
                             np.asarray(dec_W_h), np.asarray(dec_b_h),
                             np.asarray(dec_W_out), np.asarray(dec_b_out))

    enc_nc = _get_module("enc", lambda: _build_mlp_module(PHYS, LAT, TOK_ENC, "pair"))
    dec_nc = _get_module("dec", lambda: _build_mlp_module(LAT, PHYS, TOK_DEC, "f32r"))

    # ---- launch 1: encoder ----
    in_maps = []
    for c in range(N_CORES):
        xs = x[c * BPC:(c + 1) * BPC].reshape(TOK_ENC, PHYS)
        in_maps.append({"xt": np.ascontiguousarray(xs.T), "wpack": enc_pack})
    enc_res = _run_spmd(enc_nc, in_maps)
    y = np.concatenate(
        [enc_res[c]["yt"].T.reshape(BPC, T, LAT) for c in range(N_CORES)], axis=0)

    # ---- host: EDMD chain, replicated verbatim from the reference ----
    import jax
    import jax.numpy as jnp
    cpu = jax.devices("cpu")[0]
    with jax.default_device(cpu):
        yj = jnp.asarray(y)
        Y = jnp.swapaxes(yj, 1, 2)
        Y_m = Y[:, :, :-1]
        Y_p = Y[:, :, 1:]
        U, sig, Vh = jnp.linalg.svd(Y_m, full_matrices=False)
        A = ((Y_p @ jnp.swapaxes(Vh, -1, -2)) * (1.0 / sig)[:, None, :]) \
            @ jnp.swapaxes(U, -1, -2)
        evals, evecs = jnp.linalg.eig(A)
        phi = jnp.linalg.solve(evecs, Y_m.astype(evecs.dtype))
        y0 = phi[:, :, 0]
        powers = evals[:, None, :] ** jnp.arange(P_STEPS)[None, :, None]
        y_adv = jnp.real(jnp.einsum('blm,bkm->bkl', evecs, powers * y0[:, None, :]))
        evals = np.asarray(evals)
        evecs = np.asarray(evecs)
        phi = np.asarray(phi)
        y_adv = np.asarray(y_adv)

    # ---- launch 2: decoder on [y ; y_adv] ----
    in_maps = []
    for c in range(N_CORES):
        z = np.empty((TOK_DEC, LAT), np.float32)
        z[:TOK_ENC] = y[c * BPC:(c + 1) * BPC].reshape(TOK_ENC, LAT)
        z[TOK_ENC:] = y_adv[c * BPC:(c + 1) * BPC].reshape(TOK_ENC, LAT)
        in_maps.append({"xt": np.ascontiguousarray(z.T), "wpack": dec_pack})
    dec_res = _run_spmd(dec_nc, in_maps)
    x_ae = np.concatenate(
        [dec_res[c]["yt"][:, :TOK_ENC].T.reshape(BPC, T, PHYS)
         for c in range(N_CORES)], axis=0)
    x_adv = np.concatenate(
        [dec_res[c]["yt"][:, TOK_ENC:].T.reshape(BPC, P_STEPS, PHYS)
         for c in range(N_CORES)], axis=0)

    return (y, x_ae, x_adv, y_adv, evals, evecs, phi)
